# revision 1
# baseline (speedup 1.0000x reference)
"""FusionDeepONet trunk kernel for 8 Trainium2 NeuronCores.

Strategy:
 - Branch tower (16x128 MLP) is tiny -> computed on host in float64.
 - Rowdy activation tanh(z) + sum_k a_k sin(k z) (k=1..3) is computed from
   only 3 ACT passes per layer: t=tanh(z+b), s=sin(z+b), c=cos(z+b) via
   sin(z+b+pi/2).  Then sin2z = 2sc, sin3z = 4sc^2 - s, so with u=s*c,
   v=u*c (2 DVE tensor_tensor ops):
     rowdy*S = S.t + S(a1-a3).s + 2Sa2.u + 4Sa3.v
 - The per-(layer,geometry) coefficient vectors are folded into row-scaled
   copies of the next layer's weight matrix, so each layer transition is 4
   accumulating PE matmuls over the feature maps {t,s,u,v}; no vector
   broadcast work at all.  The final layer folds final_W AND the einsum
   with ZL into per-geometry [128,4] matrices G_k.
 - Data parallel: 2 geometries per core; points padded 20000->20480 and
   processed in 10 tiles of 2048 (4 PSUM banks per z tile).
"""

import os
import sys

sys.path.insert(0, "/opt/trn_rl_repo")

import numpy as np

B, NPTS, H, O, L, PDIM, CDIM = 16, 20000, 128, 4, 6, 8, 3
K = 3
NCORES = 8
GEOMS = B // NCORES          # geometries per core
NT = int(os.environ.get("KERNEL_NT", "512"))   # points per tile
NPAD = 20480                 # padded points per geometry
TILES = NPAD // NT           # tiles per geometry
CH = 512                     # psum chunk (max fp32 matmul free dim)
NCH = NT // CH
TG = int(os.environ.get("KERNEL_TG", "4"))     # tiles interleaved per group
PS_BUFS = int(os.environ.get("KERNEL_PS_BUFS", str(2 * TG * (NT // CH))))
FEAT_BUFS = int(os.environ.get("KERNEL_FEAT_BUFS", str(TG + 2)))
AUX_BUFS = int(os.environ.get("KERNEL_AUX_BUFS", str(TG + 1)))

# matmul precision knob: "f32r" (1 cyc/row, reduced-precision multiply) or
# "f32" (4 cyc/row, exact fp32)
MM_DTYPE = os.environ.get("MM_DTYPE", "f32r")

_PROGRAM_CACHE = {}


_ACT_TABLES_PATCHED = False


def _patch_act_table_choice():
    """Steer the ACT table-set chooser to `silu_and_others`, the one set that
    contains BOTH Tanh and Sin.  The default greedy chooser resolves Tanh to
    `exp_and_others` and Sin to `trig_and_small`, which forces a ~2.7us table
    reload on every one of the 360 activation passes.  We advertise Tanh/Sin
    only from the set that really serves both, so exactly one load is emitted.
    Set ids / ordering are unchanged; the hardware genuinely computes tanh and
    sin from that set."""
    global _ACT_TABLES_PATCHED
    if _ACT_TABLES_PATCHED:
        return
    import concourse.bacc as bacc
    from concourse import mybir

    orig = bacc.get_activation_tables

    def patched(arch):
        tabs = dict(orig(arch))
        both = {
            name
            for name, fns in tabs.items()
            if mybir.ActivationFunctionType.Sin in fns
            and mybir.ActivationFunctionType.Tanh in fns
        }
        if not both:
            return tabs
        keep = "silu_and_others" if "silu_and_others" in both else next(iter(both))
        out = {}
        for name, fns in tabs.items():
            if name != keep:
                fns = fns - {
                    mybir.ActivationFunctionType.Sin,
                    mybir.ActivationFunctionType.Tanh,
                }
            out[name] = fns
        return out

    bacc.get_activation_tables = patched
    _ACT_TABLES_PATCHED = True


def _build_program(mm_dtype: str, reps: int = 1):
    import concourse.bacc as bacc
    import concourse.tile as tile
    from concourse import mybir

    _patch_act_table_choice()

    f32 = mybir.dt.float32
    mm_dt = mybir.dt.float32r if mm_dtype == "f32r" else mybir.dt.float32
    Tanh = mybir.ActivationFunctionType.Tanh
    Sin = mybir.ActivationFunctionType.Sin
    INV2PI = float(1.0 / (2.0 * np.pi))
    TWOPI = float(2.0 * np.pi)
    MAGIC = float(1.5 * 2.0**23)

    nc = bacc.Bacc("TRN2", target_bir_lowering=False, debug=False)

    x_d = nc.dram_tensor("x", [GEOMS, CDIM + 1, NPAD], mm_dt, kind="ExternalInput").ap()
    wt_d = nc.dram_tensor(
        "wt", [H, GEOMS, L - 1, 4, H], mm_dt, kind="ExternalInput"
    ).ap()
    g_d = nc.dram_tensor("g", [H, GEOMS, 4, O], mm_dt, kind="ExternalInput").ap()
    w0_d = nc.dram_tensor("w0", [CDIM + 1, H], mm_dt, kind="ExternalInput").ap()
    bt_d = nc.dram_tensor("bt", [H, L], f32, kind="ExternalInput").ap()
    bh_d = nc.dram_tensor("bh", [H, L], f32, kind="ExternalInput").ap()
    ra_d = nc.dram_tensor("ra", [H, 1], f32, kind="ExternalInput").ap()
    out_d = nc.dram_tensor("out", [GEOMS, O, NPAD], f32, kind="ExternalOutput").ap()

    with tile.TileContext(nc) as tc:
        with (
            tc.tile_pool(name="consts", bufs=1) as consts,
            tc.tile_pool(name="xin", bufs=TG + 2) as xin,
            tc.tile_pool(name="feat", bufs=FEAT_BUFS) as feat,
            tc.tile_pool(name="aux", bufs=AUX_BUFS) as aux,
            tc.tile_pool(name="ps", bufs=PS_BUFS, space="PSUM") as ps,
        ):
            wt_sb = consts.tile([H, GEOMS, L - 1, 4, H], mm_dt)
            nc.sync.dma_start(out=wt_sb[:], in_=wt_d[:])
            g_sb = consts.tile([H, GEOMS, 4, O], mm_dt)
            nc.sync.dma_start(out=g_sb[:], in_=g_d[:])
            w0_sb = consts.tile([CDIM + 1, H], mm_dt)
            nc.sync.dma_start(out=w0_sb[:], in_=w0_d[:])
            bt_sb = consts.tile([H, L], f32)
            nc.sync.dma_start(out=bt_sb[:], in_=bt_d[:])
            bh_sb = consts.tile([H, L], f32)
            nc.sync.dma_start(out=bh_sb[:], in_=bh_d[:])
            ra_sb = consts.tile([H, 1], f32)
            nc.sync.dma_start(out=ra_sb[:], in_=ra_d[:])

            all_tiles = [
                (g, jt) for g in range(GEOMS) for jt in range(TILES)
            ]

            import contextlib

            rep_loop = (
                tc.For_i(0, reps, 1) if reps > 1 else contextlib.nullcontext()
            )
            with rep_loop:
                _emit_tiles(
                    nc, tc, mybir, all_tiles, xin, feat, aux, ps,
                    x_d, out_d, wt_sb, g_sb, w0_sb, bt_sb, bh_sb, ra_sb,
                    f32, mm_dt, Tanh, Sin, INV2PI, TWOPI, MAGIC,
                )
    nc.finalize()
    return nc


def _emit_tiles(nc, tc, mybir, all_tiles, xin, feat, aux, ps,
                x_d, out_d, wt_sb, g_sb, w0_sb, bt_sb, bh_sb, ra_sb,
                f32, mm_dt, Tanh, Sin, INV2PI, TWOPI, MAGIC):
            for g0 in range(0, len(all_tiles), TG):
                grp = all_tiles[g0 : g0 + TG]
                G = len(grp)
                # ---- layer-0 preactivation for every tile in the group ----
                zs = [None] * G
                for ix, (g, jt) in enumerate(grp):
                    n0 = jt * NT
                    x_t = xin.tile([CDIM + 1, NT], mm_dt, tag="x")
                    nc.sync.dma_start(out=x_t[:], in_=x_d[g, :, n0 : n0 + NT])
                    z = ps.tile([H, NT], f32, tag="z")
                    for c in range(NCH):
                        cs = slice(c * CH, (c + 1) * CH)
                        nc.tensor.matmul(
                            z[:, cs], lhsT=w0_sb[:], rhs=x_t[:, cs],
                            start=True, stop=True,
                        )
                    zs[ix] = z

                for i in range(L):
                    # Sin LUT valid only on |arg| <= ~3.79: layer 0 args
                    # reach ~10, so range-reduce into [-pi, pi] (magic-
                    # number round); later layers are in range.  sin2/sin3
                    # come from h = sin(arg/2), whose square is immune to
                    # the 2*pi*k fold parity.
                    sin_srcs = [None] * G
                    if i == 0:
                        for ix in range(G):
                            y_t = aux.tile([H, NT], f32, tag="y")
                            nc.vector.tensor_scalar(
                                y_t[:], zs[ix][:], INV2PI, ra_sb[:, 0:1],
                                op0=mybir.AluOpType.mult,
                                op1=mybir.AluOpType.add,
                            )
                            r_t = aux.tile([H, NT], f32, tag="r")
                            nc.vector.tensor_scalar(
                                r_t[:], y_t[:], MAGIC, -TWOPI,
                                op0=mybir.AluOpType.subtract,
                                op1=mybir.AluOpType.mult,
                            )
                            nc.vector.tensor_add(r_t[:], zs[ix][:], r_t[:])
                            sin_srcs[ix] = r_t
                    else:
                        sin_srcs = zs

                    hts, sts, tts, hhs, wts, vts = ([None] * G for _ in range(6))
                    for ix in range(G):
                        h_t = feat.tile([H, NT], f32, tag="h")
                        s_t = feat.tile([H, NT], mm_dt, tag="s")
                        t_t = feat.tile([H, NT], mm_dt, tag="t")
                        nc.scalar.activation(
                            h_t[:], sin_srcs[ix][:], Sin,
                            scale=0.5, bias=bh_sb[:, i : i + 1],
                        )
                        nc.scalar.activation(
                            s_t[:], sin_srcs[ix][:], Sin,
                            bias=bt_sb[:, i : i + 1],
                        )
                        nc.scalar.activation(
                            t_t[:], zs[ix][:], Tanh, bias=bt_sb[:, i : i + 1]
                        )
                        hts[ix], sts[ix], tts[ix] = h_t, s_t, t_t
                    for ix in range(G):
                        hh_t = feat.tile([H, NT], f32, tag="hh")
                        nc.gpsimd.tensor_mul(hh_t[:], hts[ix][:], hts[ix][:])
                        hhs[ix] = hh_t
                    for ix in range(G):
                        w_t = feat.tile([H, NT], mm_dt, tag="w")
                        v_t = feat.tile([H, NT], mm_dt, tag="v")
                        nc.vector.tensor_mul(w_t[:], sts[ix][:], hhs[ix][:])
                        nc.vector.tensor_mul(v_t[:], w_t[:], hhs[ix][:])
                        wts[ix], vts[ix] = w_t, v_t

                    featmaps = [
                        (tts[ix], sts[ix], wts[ix], vts[ix]) for ix in range(G)
                    ]
                    if i < L - 1:
                        z2s = [ps.tile([H, NT], f32, tag="z", name=f"z_{g0}_{i}_{ixx}") for ixx in range(G)]
                        for ix in range(G):
                            g = grp[ix][0]
                            for c in range(NCH):
                                cs = slice(c * CH, (c + 1) * CH)
                                for k in range(4):
                                    nc.tensor.matmul(
                                        z2s[ix][:, cs],
                                        lhsT=wt_sb[:, g, i, k, :],
                                        rhs=featmaps[ix][k][:, cs],
                                        start=(k == 0), stop=(k == 3),
                                    )
                        zs = z2s
                    else:
                        for ix in range(G):
                            g, jt = grp[ix]
                            n0 = jt * NT
                            o_t = ps.tile([O, NT], f32, tag="z")
                            for c in range(NCH):
                                cs = slice(c * CH, (c + 1) * CH)
                                for k in range(4):
                                    nc.tensor.matmul(
                                        o_t[:, cs],
                                        lhsT=g_sb[:, g, k, :],
                                        rhs=featmaps[ix][k][:, cs],
                                        start=(k == 0), stop=(k == 3),
                                    )
                            o_sb = aux.tile([O, NT], f32, tag="o")
                            nc.vector.tensor_copy(o_sb[:], o_t[:])
                            nc.sync.dma_start(
                                out=out_d[g, :, n0 : n0 + NT], in_=o_sb[:]
                            )


def _get_program(mm_dtype: str):
    if mm_dtype not in _PROGRAM_CACHE:
        _PROGRAM_CACHE[mm_dtype] = _build_program(mm_dtype)
    return _PROGRAM_CACHE[mm_dtype]


LAST_EXEC_NS = None
LAST_RESULTS = None


def _prepare(
    coords,
    sdf,
    params,
    branch_W0,
    branch_Wr,
    branch_b,
    branch_Wout,
    branch_bout,
    trunk_W0,
    trunk_Wr,
    trunk_b,
    rowdy_a,
    final_W,
    final_b,
):
    f8 = np.float64

    # ---- branch tower on host (tiny) ----
    h = np.tanh(np.asarray(params, f8) @ np.asarray(branch_W0, f8) + np.asarray(branch_b, f8)[0])
    hiddens = [h]
    for i in range(1, L):
        h = np.tanh(h @ np.asarray(branch_Wr, f8)[i - 1] + np.asarray(branch_b, f8)[i])
        hiddens.append(h)
    branch_out = h @ np.asarray(branch_Wout, f8) + np.asarray(branch_bout, f8)
    S = [hiddens[0]]
    for i in range(1, L):
        S.append(hiddens[i] + S[-1])
    ZL = branch_out.reshape(B, O, H)

    # ---- fold rowdy coefficients + fusion scales into weights ----
    a = np.asarray(rowdy_a, f8)  # (L, K, H)
    # features: t=tanh, s=sin, u=sin*cos, v=sin*cos^2
    #   rowdy = t + a1 s + a2 (2u) + a3 (4v - s)
    # basis {t, s, w=s*h^2, ww=w*h^2} with h = sin(arg/2):
    #   sin2 = 2s - 4w,  sin3 = 3s - 16w + 16ww
    C = np.empty((L, 4, B, H), f8)
    for i in range(L):
        C[i, 0] = S[i]
        C[i, 1] = S[i] * (a[i, 0] + 2.0 * a[i, 1] + 3.0 * a[i, 2])
        C[i, 2] = S[i] * (-4.0 * a[i, 1] - 16.0 * a[i, 2])
        C[i, 3] = S[i] * (16.0 * a[i, 2])

    Wr = np.asarray(trunk_Wr, f8)  # (L-1, H, H)
    fW = np.asarray(final_W, f8)   # (H, H)
    # transitions: z_{i+1} = sum_k (diag(C[i,k,b]) Wr[i]).T f_k + b_{i+1}
    Wt = np.einsum("ikbh,ihm->bikhm", C[: L - 1], Wr)          # (B, L-1, 4, H, H)
    # final fold + einsum: out[o,n] = sum_k G[b,k].T f_k(z5)
    #   G[b,k] = diag(C[5,k,b]) @ final_W @ ZL[b].T    -> (H, O)
    G = np.einsum("kbh,hm,bom->bkho", C[L - 1], fW, ZL)        # (B, 4, H, O)
    obias = np.einsum("boh,h->bo", ZL, np.asarray(final_b, f8))  # (B, O)

    # ---- device-layout arrays ----
    x = np.concatenate(
        [np.asarray(coords, np.float32), np.asarray(sdf, np.float32)], axis=-1
    )  # (B, NPTS, 4)
    x = np.ascontiguousarray(np.transpose(x, (0, 2, 1)))  # (B, 4, NPTS)
    xpad = np.zeros((B, CDIM + 1, NPAD), np.float32)
    xpad[:, :, :NPTS] = x

    wt_all = np.ascontiguousarray(
        np.transpose(Wt, (3, 0, 1, 2, 4)).astype(np.float32)
    )  # (H, B, L-1, 4, H)
    g_all = np.ascontiguousarray(
        np.transpose(G, (2, 0, 1, 3)).astype(np.float32)
    )  # (H, B, 4, O)
    w0 = np.ascontiguousarray(np.asarray(trunk_W0, np.float32))  # (4, H)
    bt = np.ascontiguousarray(np.asarray(trunk_b, np.float32).T)  # (H, L)
    bh = np.ascontiguousarray((np.asarray(trunk_b, np.float32) / 2.0).astype(np.float32).T)
    # range-reduction add-constant: b0/(2 pi) + magic rounding constant
    ra = np.ascontiguousarray(
        (np.asarray(trunk_b, np.float64)[0] / (2.0 * np.pi) + 1.5 * 2.0**23)
        .astype(np.float32)
        .reshape(H, 1)
    )

    in_maps = []
    for core in range(NCORES):
        gsel = slice(core * GEOMS, (core + 1) * GEOMS)
        in_maps.append(
            {
                "x": np.ascontiguousarray(xpad[gsel]),
                "wt": np.ascontiguousarray(wt_all[:, gsel]),
                "g": np.ascontiguousarray(g_all[:, gsel]),
                "w0": w0,
                "bt": bt,
                "bh": bh,
                "ra": ra,
            }
        )

    return in_maps, obias


def prepare_in_maps(**inputs):
    return _prepare(**inputs)[0]


def kernel(**inputs):
    global LAST_EXEC_NS, LAST_RESULTS
    from concourse.bass_utils import run_bass_kernel_spmd

    in_maps, obias = _prepare(**inputs)
    nc = _get_program(MM_DTYPE)
    trace = bool(int(os.environ.get("KERNEL_TRACE", "0")))
    res = run_bass_kernel_spmd(nc, in_maps, list(range(NCORES)), trace=trace)
    LAST_EXEC_NS = res.exec_time_ns
    LAST_RESULTS = res

    outs = np.concatenate([res.results[c]["out"] for c in range(NCORES)], axis=0)
    # (B, O, NPAD) -> (B, NPTS, O)
    out = np.transpose(outs[:, :, :NPTS], (0, 2, 1)).astype(np.float64)
    out += obias[:, None, :]
    return out.astype(np.float32)

